# revision 7
# baseline (speedup 1.0000x reference)
"""BAM self-attention block (B=8, C=256, H=W=64) on 8 TRN2 NeuronCores.

Sharding: data-parallel over batch - one batch element per core; the small
1x1-conv weights are replicated to every core.

Per-core algorithm (x is [C=256, N=4096]; all matmuls on the PE, bf16
operands with fp32 PSUM accumulation):
  q = Wq x + bq   [32, N] replicated to 4 PE row groups via column-replicated
                  transposed weights (one matmul writes all 4 replicas)
  k = Wk x + bk   [32, N] likewise
  vT = (Wv x)^T   [N, 256] (bias bv folded into the output residual, since
                  softmax rows sum to 1)
  S^T[n, m] = sum_c k[c,n] q[c,m]  computed directly transposed so the second
              matmul's contraction (over n) lies on partitions; 4 key-blocks
              run concurrently via PE row-tiling (K=32 each) into one 4-bank
              PSUM tile.
  P^T = exp(S^T)  one whole-tile ACT pass -> bf16 (no row-max subtraction:
                  |S| < 45 so fp32 exp cannot overflow; softmax
                  shift-invariance makes the result exact)
  s[m] = sum_n P^T[n, m]  4 col-tiled M=1 ones-matmuls (concurrent) + a K=4
                  reduce+broadcast matmul, then a fast DVE reciprocal
  out[c, m] = sum_n vT[n, c] P^T[n, m]  accumulated in PSUM over all 32 blocks
  y = gamma/s * out + (x + gamma*bv)

v2 software pipeline (3 phases deep, one query chunk mc per phase):
  phase(mc): S^T+exp+s-sums of chunk mc interleaved with the out-matmul
  blocks of chunk mc-1 and the normalize/store tail of chunk mc-2.  The
  projections (q/k/v per chunk) are slotted into phase(0), which has no
  out-matmuls yet, so the PE never drains during the former projection
  prologue.  The last chunk's out-matmuls run column-half-major so its own
  tail overlaps the second half.  q/k bias-adds run on the DVE (tensor_scalar
  with a per-partition bias column) so the scalar engine runs exp only.
"""
import sys
import numpy as np

for p in ("/opt/trn_rl_repo",):
    if p not in sys.path:
        sys.path.insert(0, p)

B, C, H, W = 8, 256, 64, 64
N = H * W          # 4096
CK = C // 8        # 32
NB = N // 128      # 32 key blocks
MC = N // 512      # 8 query chunks
NG = NB // 4       # 8 groups of 4 key blocks

_NC_CACHE = {}


def _build_nc():
    import concourse.mybir as mybir
    import concourse.tile as tile
    from concourse import bacc
    from concourse.bass import ds

    f32, f32r, bf16 = mybir.dt.float32, mybir.dt.float32r, mybir.dt.bfloat16
    Exp = mybir.ActivationFunctionType.Exp

    nc = bacc.Bacc("TRN2", target_bir_lowering=False, debug=False)

    x_d = nc.dram_tensor("x", [C, N], f32, kind="ExternalInput").ap()
    wq_d = nc.dram_tensor("Wq", [CK, C], f32, kind="ExternalInput").ap()
    bq_d = nc.dram_tensor("bq", [CK], f32, kind="ExternalInput").ap()
    wk_d = nc.dram_tensor("Wk", [CK, C], f32, kind="ExternalInput").ap()
    bk_d = nc.dram_tensor("bk", [CK], f32, kind="ExternalInput").ap()
    wv_d = nc.dram_tensor("Wv", [C, C], f32, kind="ExternalInput").ap()
    bv_d = nc.dram_tensor("bv", [C], f32, kind="ExternalInput").ap()
    g_d = nc.dram_tensor("gamma", [1], f32, kind="ExternalInput").ap()
    y_d = nc.dram_tensor("y", [C, N], f32, kind="ExternalOutput").ap()

    x_r = x_d.rearrange("(o p) n -> p o n", p=128)   # c = o*128 + p
    y_r = y_d.rearrange("(o p) n -> p o n", p=128)

    with tile.TileContext(nc) as tc:
        with tc.tile_pool(name="const", bufs=1) as const, \
             tc.tile_pool(name="big", bufs=1) as big, \
             tc.tile_pool(name="work", bufs=4) as work, \
             tc.tile_pool(name="ptp", bufs=10) as ptp, \
             tc.tile_pool(name="ps_st", bufs=1, space="PSUM") as ps_st, \
             tc.tile_pool(name="ps_out", bufs=2, space="PSUM") as ps_out, \
             tc.tile_pool(name="ps_misc", bufs=1, space="PSUM") as ps_misc:

            # ---------- constants / weights (natural layout, transposed on PE) ----------
            from concourse.masks import make_identity
            ident = const.tile([128, 128], f32, tag="ident")
            make_identity(nc, ident[:])

            # biases: bq/bk replicated to all 4 row groups (applied on DVE)
            bq4 = const.tile([128, 1], f32, tag="bq4")
            bk4 = const.tile([128, 1], f32, tag="bk4")
            for j in range(4):
                nc.gpsimd.dma_start(bq4[32 * j:32 * (j + 1), :], bq_d[:, None])
                nc.gpsimd.dma_start(bk4[32 * j:32 * (j + 1), :], bk_d[:, None])
            bv2 = const.tile([128, 2], f32, tag="bv2")
            nc.gpsimd.dma_start(bv2[:], bv_d.rearrange("(o p) -> p o", p=128))
            g_col = const.tile([128, 1], f32, tag="gcol")
            nc.gpsimd.dma_start(g_col[:], g_d[None, :].to_broadcast([128, 1]))

            ones1 = const.tile([128, 1], bf16, tag="ones1")
            nc.any.memset(ones1[:], 1.0)
            ones4_raw = work.tile([4, 128], f32, tag="o4raw")
            nc.any.memset(ones4_raw[:], 1.0)
            ones4 = const.tile([4, 128], f32r, tag="ones4")
            nc.vector.tensor_copy(ones4[:], ones4_raw[:])

            gbv = const.tile([128, 2], f32, tag="gbv")
            nc.vector.tensor_scalar_mul(gbv[:], bv2[:], g_col[:])

            # Wq/Wk [32, 256] natural -> transpose chunks -> wqT4/wkT4
            wq_nat = work.tile([CK, C], f32, tag="wnat")
            nc.sync.dma_start(wq_nat[:], wq_d[:])
            wk_nat = work.tile([CK, C], f32, tag="wnat")
            nc.sync.dma_start(wk_nat[:], wk_d[:])
            # wqT4/wkT4: transposed weights with the 32 columns replicated 4x,
            # so one matmul yields q replicated across all 4 PE row groups
            wqT4 = const.tile([128, 2, 128], bf16, tag="wqT4")
            wkT4 = const.tile([128, 2, 128], bf16, tag="wkT4")
            for nat, dstw in ((wq_nat, wqT4), (wk_nat, wkT4)):
                for o in range(2):
                    tp = ps_out.tile([128, CK], f32, tag="out")
                    nc.tensor.transpose(tp[:], nat[:, ds(128 * o, 128)],
                                        ident[0:CK, 0:CK])
                    for j in range(4):
                        nc.vector.tensor_copy(dstw[:, o, ds(32 * j, 32)], tp[:])

            # Wv [256, 256] natural -> 4 transposed blocks -> wvT [128, 2, 256]
            wv_nat = work.tile([128, 2, C], f32, tag="wvnat")
            wv_n = wv_d.rearrange("(o p) c -> p o c", p=128)
            for o in range(2):
                nc.sync.dma_start(wv_nat[:, o], wv_n[:, o])
            wvT = const.tile([128, 2, C], bf16, tag="wvT")
            for o_c in range(2):
                for o_co in range(2):
                    tp = ps_out.tile([128, 128], f32, tag="out")
                    nc.tensor.transpose(tp[:], wv_nat[:, o_co, ds(128 * o_c, 128)],
                                        ident[:])
                    nc.vector.tensor_copy(wvT[:, o_c, ds(128 * o_co, 128)], tp[:])

            # x chunks: issue every load up front, weights having gone first;
            # spread over both HWDGE queues so projections are never starved
            xs = big.tile([128, 2, N], f32, tag="xs")
            for mc in range(MC):
                ms = ds(512 * mc, 512)
                eng = nc.sync if mc < 4 else nc.scalar
                eng.dma_start(xs[:, :, ms], x_r[:, :, ms])

            # ---------- per-chunk tiles ----------
            xr = big.tile([128, 2, N], bf16, tag="xr")
            q4c = [big.tile([128, 512], bf16, tag=f"q4_{i}", name=f"q4_{i}")
                   for i in range(MC)]
            k4c = [big.tile([128, 512], bf16, tag=f"k4_{i}", name=f"k4_{i}")
                   for i in range(MC)]
            vTc = [big.tile([128, 4, C], bf16, tag=f"vT_{i}", name=f"vT_{i}")
                   for i in range(MC)]

            def proj(mc):
                """Emit chunk mc's projections: cast, q, k (DVE bias-add),
                vT blocks, and the chunk's residual base xs += gamma*bv."""
                ms = ds(512 * mc, 512)
                nc.vector.tensor_copy(xr[:, :, ms], xs[:, :, ms])
                for w_t, b4, dst in ((wqT4, bq4, q4c[mc]), (wkT4, bk4, k4c[mc])):
                    pp = ps_out.tile([128, 512], f32, tag="out")
                    for o in range(2):
                        nc.tensor.matmul(pp[:], w_t[:, o, :], xr[:, o, ms],
                                         start=(o == 0), stop=(o == 1))
                    nc.vector.tensor_scalar_add(dst[:], pp[:], b4[:])
                for nb in range(4 * mc, 4 * mc + 4):
                    pv = ps_out.tile([128, C], f32, tag="out")
                    for o in range(2):
                        nc.tensor.matmul(pv[:], xr[:, o, ds(128 * nb, 128)],
                                         wvT[:, o, :], start=(o == 0), stop=(o == 1))
                    nc.vector.tensor_copy(vTc[mc][:, nb - 4 * mc, :], pv[:])
                for o in range(2):
                    nc.vector.tensor_scalar_add(xs[:, o, ms], xs[:, o, ms],
                                                gbv[:, o:o + 1])

            def st_group(mc, g):
                """Emit the 4 row-tiled S^T matmuls + whole-tile exp for group
                g of chunk mc; returns the bf16 P^T tile."""
                st = ps_st.tile([128, 2048], f32, tag="st", name=f"st_{mc}_{g}")
                for j in range(4):
                    nb = 4 * g + j
                    nc.tensor.matmul(st[:, ds(512 * j, 512)],
                                     k4c[nb // 4][32 * j:32 * (j + 1),
                                                  ds(128 * (nb % 4), 128)],
                                     q4c[mc][32 * j:32 * (j + 1), :],
                                     start=True, stop=True,
                                     tile_position=(32 * j, 0))
                pt = ptp.tile([128, 2048], bf16, tag="pt", name=f"pt_{mc}_{g}")
                nc.scalar.activation(pt[:], st[:], Exp)
                return pt

            def s_sum(mc, g, s_ps, pt):
                """4 col-tiled partition-sum matmuls for group g (accumulating
                into the chunk's s_ps bank)."""
                for j in range(4):
                    nc.tensor.matmul(s_ps[32 * j:32 * j + 1, :], ones1[:],
                                     pt[:, ds(512 * j, 512)],
                                     start=(g == 0), stop=(g == NG - 1),
                                     tile_position=(0, 32 * j))

            def out_mms(mc, g, out_ps, pt, cc_list=(0, 1)):
                """8 (or 4) out-matmul accumulations for group g of chunk mc."""
                for j in range(4):
                    nb = 4 * g + j
                    for cc in cc_list:
                        nc.tensor.matmul(out_ps[cc][:],
                                         vTc[nb // 4][:, nb % 4,
                                                      ds(128 * cc, 128)],
                                         pt[:, ds(512 * j, 512)],
                                         start=(g == 0 and j == 0),
                                         stop=(g == NG - 1 and j == 3))

            def s_chain_a(mc, s4c):
                """s partial gather + broadcast matmul (PE part)."""
                s4_sb = work.tile([4, 512], f32r, tag="s4", name=f"s4_{mc}")
                nc.gpsimd.dma_start(s4_sb[:], s4c[0:97:32, :])
                srep_ps = ps_misc.tile([128, 512], f32, tag="srep",
                                       name=f"srep_{mc}")
                nc.tensor.matmul(srep_ps[:], ones4[:], s4_sb[:],
                                 start=True, stop=True)
                return srep_ps

            def s_chain_b(mc, srep_ps):
                """reciprocal + gamma scale -> r_rep (DVE part)."""
                r_rep = work.tile([128, 512], f32, tag="rrep", name=f"rr_{mc}")
                nc.vector.reciprocal_approx_fast(r_rep[:], srep_ps[:])
                nc.vector.tensor_scalar_mul(r_rep[:], r_rep[:], g_col[:])
                return r_rep

            def tail(mc, out_sb, r_rep, cc_list=(0, 1)):
                """normalize + residual + store for chunk mc (DVE + DMA)."""
                for cc in cc_list:
                    y_sb = work.tile([128, 512], f32, tag="y",
                                     name=f"y_{mc}_{cc}")
                    for h in range(2):
                        hs = ds(256 * h, 256)
                        ys = ds(512 * mc + 256 * h, 256)
                        t_sb = work.tile([128, 256], f32, tag="t")
                        nc.vector.tensor_mul(t_sb[:], out_sb[cc][:, hs],
                                             r_rep[:, hs])
                        nc.vector.tensor_add(y_sb[:, hs], t_sb[:],
                                             xs[:, cc, ys])
                        nc.sync.dma_start(y_r[:, cc, ys], y_sb[:, hs])

            # ---------- prologue: first projections + first S^T ----------
            proj(0)
            proj(1)

            # pipeline state: keyed by chunk
            pts = {}            # (mc, g) -> pt tile (alive one full phase)
            s_ps_cur = None
            s4c_of = {}
            srep_of = {}
            rrep_of = {}
            outps_of = {}
            outsb_of = {}

            # ---------- phases ----------
            for ph in range(MC + 1):
                mc = ph                # chunk whose S^T/exp/s-sums run now
                po = ph - 1            # chunk whose out-matmuls run now
                pk = ph - 2            # chunk whose tail runs now

                if mc < MC:
                    s_ps_cur = ps_misc.tile([128, 512], f32, tag="sacc",
                                            name=f"sacc_{mc}")

                if po >= 0:
                    outps_of[po] = [ps_out.tile([128, 512], f32, tag="out",
                                                name=f"out_{po}_{cc}")
                                    for cc in range(2)]

                for g in range(NG):
                    # this chunk's S^T + exp
                    if mc < MC:
                        pts[(mc, g)] = st_group(mc, g)

                    # tail-of-pk bookkeeping (DVE-only, PE unaffected):
                    # out_ps -> SBUF copies first so the banks recycle
                    if g == 0 and pk >= 0:
                        ob = []
                        for cc in range(2):
                            o_sb = work.tile([128, 512], f32, tag=f"ob{cc}",
                                             name=f"ob_{pk}_{cc}")
                            nc.vector.tensor_copy(o_sb[:],
                                                  outps_of[pk][cc][:])
                            ob.append(o_sb)
                        outsb_of[pk] = ob
                    if g == 1:
                        if po >= 0:
                            rrep_of[po] = s_chain_b(po, srep_of[po])
                        if pk >= 0:
                            tail(pk, outsb_of[pk], rrep_of[pk])
                            del outsb_of[pk], rrep_of[pk], outps_of[pk]

                    # out-matmuls of the previous chunk (last chunk runs
                    # cc-major in the epilogue phase so its tail overlaps)
                    if po >= 0:
                        if po < MC - 1:
                            out_mms(po, g, outps_of[po], pts.pop((po, g)))
                        else:
                            cc = g // 4
                            gg = g % 4 * 2
                            for g2 in (gg, gg + 1):
                                out_mms(po, g2, outps_of[po],
                                        pts[(po, g2)], cc_list=(cc,))
                            if cc == 0 and g == 3:
                                # first half done: normalize + store it early
                                o_sb = work.tile([128, 512], f32, tag="ob0",
                                                 name=f"ob_{po}_0")
                                nc.vector.tensor_copy(o_sb[:],
                                                      outps_of[po][0][:])
                                outsb_of[po] = [o_sb]
                                tail(po, outsb_of[po], rrep_of[po],
                                     cc_list=(0,))

                    # previous chunk's s gather + broadcast (the srep matmul
                    # goes after this slot's out-matmuls so its DMA-gather
                    # latency never head-blocks the PE queue)
                    if g == 0 and po >= 0:
                        srep_of[po] = s_chain_a(po, s4c_of[po])

                    # projections ride in phase 0's spare PE slots
                    if ph == 0 and g + 2 < MC:
                        proj(g + 2)

                    # s partial sums for this chunk (lag 1 group)
                    if mc < MC and g >= 1:
                        s_sum(mc, g - 1, s_ps_cur, pts[(mc, g - 1)])

                if mc < MC:
                    s_sum(mc, NG - 1, s_ps_cur, pts[(mc, NG - 1)])
                    s4c = work.tile([128, 512], f32r, tag="s4c",
                                    name=f"s4c_{mc}")
                    nc.vector.tensor_copy(s4c[:], s_ps_cur[:])
                    s4c_of[mc] = s4c

            # ---------- epilogue: last chunk's second-half tail ----------
            ml = MC - 1
            o_sb = work.tile([128, 512], f32, tag="ob1", name=f"ob_{ml}_1")
            nc.vector.tensor_copy(o_sb[:], outps_of[ml][1][:])
            tail(ml, {1: o_sb}, rrep_of[ml], cc_list=(1,))

    nc.compile()
    return nc


def kernel(x, Wq, bq, Wk, bk, Wv, bv, gamma):
    from concourse import bass_utils

    if "nc" not in _NC_CACHE:
        _NC_CACHE["nc"] = _build_nc()
    nc = _NC_CACHE["nc"]

    x = np.ascontiguousarray(np.asarray(x, dtype=np.float32))
    shared = {
        "Wq": np.ascontiguousarray(np.asarray(Wq, dtype=np.float32)),
        "bq": np.ascontiguousarray(np.asarray(bq, dtype=np.float32)),
        "Wk": np.ascontiguousarray(np.asarray(Wk, dtype=np.float32)),
        "bk": np.ascontiguousarray(np.asarray(bk, dtype=np.float32)),
        "Wv": np.ascontiguousarray(np.asarray(Wv, dtype=np.float32)),
        "bv": np.ascontiguousarray(np.asarray(bv, dtype=np.float32)),
        "gamma": np.ascontiguousarray(np.asarray(gamma, dtype=np.float32)),
    }
    in_maps = [dict(shared, x=np.ascontiguousarray(x[i].reshape(C, N)))
               for i in range(B)]

    res = bass_utils.run_bass_kernel_spmd(nc, in_maps, core_ids=list(range(B)))
    y = np.stack([res.results[i]["y"] for i in range(B)], axis=0)
    return y.reshape(B, C, H, W).astype(np.float32)


if __name__ == "__main__":
    rng = np.random.default_rng(0)
    ins = {
        "x": rng.standard_normal((B, C, H, W), dtype=np.float32),
        "Wq": rng.standard_normal((CK, C), dtype=np.float32) / 16,
        "bq": rng.standard_normal((CK,), dtype=np.float32) * 0.01,
        "Wk": rng.standard_normal((CK, C), dtype=np.float32) / 16,
        "bk": rng.standard_normal((CK,), dtype=np.float32) * 0.01,
        "Wv": rng.standard_normal((C, C), dtype=np.float32) / 16,
        "bv": rng.standard_normal((C,), dtype=np.float32) * 0.01,
        "gamma": rng.standard_normal((1,), dtype=np.float32) * 0.1,
    }
    y = kernel(**ins)
    print("kernel output", y.shape, y.dtype)


# revision 11
# speedup vs baseline: 1.0096x; 1.0096x over previous
"""Baseline (v1) kernel, reconstructed for A/B clock-state testing."""
import sys
import numpy as np

for p in ("/opt/trn_rl_repo",):
    if p not in sys.path:
        sys.path.insert(0, p)

B, C, H, W = 8, 256, 64, 64
N = H * W          # 4096
CK = C // 8        # 32
NB = N // 128      # 32 key blocks
MC = N // 512      # 8 query chunks
NG = NB // 4       # 8 groups of 4 key blocks

_NC_CACHE = {}


def _build_nc():
    import concourse.mybir as mybir
    import concourse.tile as tile
    from concourse import bacc
    from concourse.bass import ds

    f32, f32r, bf16 = mybir.dt.float32, mybir.dt.float32r, mybir.dt.bfloat16
    Exp = mybir.ActivationFunctionType.Exp
    Identity = mybir.ActivationFunctionType.Identity

    nc = bacc.Bacc("TRN2", target_bir_lowering=False, debug=False)

    x_d = nc.dram_tensor("x", [C, N], f32, kind="ExternalInput").ap()
    wq_d = nc.dram_tensor("Wq", [CK, C], f32, kind="ExternalInput").ap()
    bq_d = nc.dram_tensor("bq", [CK], f32, kind="ExternalInput").ap()
    wk_d = nc.dram_tensor("Wk", [CK, C], f32, kind="ExternalInput").ap()
    bk_d = nc.dram_tensor("bk", [CK], f32, kind="ExternalInput").ap()
    wv_d = nc.dram_tensor("Wv", [C, C], f32, kind="ExternalInput").ap()
    bv_d = nc.dram_tensor("bv", [C], f32, kind="ExternalInput").ap()
    g_d = nc.dram_tensor("gamma", [1], f32, kind="ExternalInput").ap()
    y_d = nc.dram_tensor("y", [C, N], f32, kind="ExternalOutput").ap()

    x_r = x_d.rearrange("(o p) n -> p o n", p=128)   # c = o*128 + p
    y_r = y_d.rearrange("(o p) n -> p o n", p=128)

    with tile.TileContext(nc) as tc:
        with tc.tile_pool(name="const", bufs=1) as const, \
             tc.tile_pool(name="big", bufs=1) as big, \
             tc.tile_pool(name="work", bufs=4) as work, \
             tc.tile_pool(name="ptp", bufs=3) as ptp, \
             tc.tile_pool(name="ps_st", bufs=1, space="PSUM") as ps_st, \
             tc.tile_pool(name="ps_out", bufs=2, space="PSUM") as ps_out, \
             tc.tile_pool(name="ps_misc", bufs=1, space="PSUM") as ps_misc:

            from concourse.masks import make_identity
            ident = const.tile([128, 128], f32, tag="ident")
            make_identity(nc, ident[:])

            bq4 = const.tile([128, 1], f32, tag="bq4")
            bk4 = const.tile([128, 1], f32, tag="bk4")
            for j in range(4):
                nc.gpsimd.dma_start(bq4[32 * j:32 * (j + 1), :], bq_d[:, None])
                nc.gpsimd.dma_start(bk4[32 * j:32 * (j + 1), :], bk_d[:, None])
            bv2 = const.tile([128, 2], f32, tag="bv2")
            nc.gpsimd.dma_start(bv2[:], bv_d.rearrange("(o p) -> p o", p=128))
            g_col = const.tile([128, 1], f32, tag="gcol")
            nc.gpsimd.dma_start(g_col[:], g_d[None, :].to_broadcast([128, 1]))

            ones1 = const.tile([128, 1], bf16, tag="ones1")
            nc.any.memset(ones1[:], 1.0)
            ones4_raw = work.tile([4, 128], f32, tag="o4raw")
            nc.any.memset(ones4_raw[:], 1.0)
            ones4 = const.tile([4, 128], f32r, tag="ones4")
            nc.vector.tensor_copy(ones4[:], ones4_raw[:])

            gbv = const.tile([128, 2], f32, tag="gbv")
            nc.vector.tensor_scalar_mul(gbv[:], bv2[:], g_col[:])

            wq_nat = work.tile([CK, C], f32, tag="wnat")
            nc.sync.dma_start(wq_nat[:], wq_d[:])
            wk_nat = work.tile([CK, C], f32, tag="wnat")
            nc.sync.dma_start(wk_nat[:], wk_d[:])
            wqT4 = const.tile([128, 2, 128], bf16, tag="wqT4")
            wkT4 = const.tile([128, 2, 128], bf16, tag="wkT4")
            for nat, dstw in ((wq_nat, wqT4), (wk_nat, wkT4)):
                for o in range(2):
                    tp = ps_out.tile([128, CK], f32, tag="out")
                    nc.tensor.transpose(tp[:], nat[:, ds(128 * o, 128)],
                                        ident[0:CK, 0:CK])
                    for j in range(4):
                        nc.vector.tensor_copy(dstw[:, o, ds(32 * j, 32)], tp[:])

            wv_nat = work.tile([128, 2, C], f32, tag="wvnat")
            wv_n = wv_d.rearrange("(o p) c -> p o c", p=128)
            for o in range(2):
                nc.sync.dma_start(wv_nat[:, o], wv_n[:, o])
            wvT = const.tile([128, 2, C], bf16, tag="wvT")
            for o_c in range(2):
                for o_co in range(2):
                    tp = ps_out.tile([128, 128], f32, tag="out")
                    nc.tensor.transpose(tp[:], wv_nat[:, o_co, ds(128 * o_c, 128)],
                                        ident[:])
                    nc.vector.tensor_copy(wvT[:, o_c, ds(128 * o_co, 128)], tp[:])

            xs = big.tile([128, 2, N], f32, tag="xs")
            xr = big.tile([128, 2, N], bf16, tag="xr")
            q4c = [big.tile([128, 512], bf16, tag=f"q4_{i}", name=f"q4_{i}")
                   for i in range(MC)]
            k4c = [big.tile([128, 512], bf16, tag=f"k4_{i}", name=f"k4_{i}")
                   for i in range(MC)]
            vTc = [big.tile([128, 4, C], bf16, tag=f"vT_{i}", name=f"vT_{i}")
                   for i in range(MC)]
            # issue every x-chunk load up front so the DMA engine runs ahead
            # of the projection pipeline
            for mc in range(MC):
                ms = ds(512 * mc, 512)
                nc.sync.dma_start(xs[:, :, ms], x_r[:, :, ms])

            def st_group(mc, g):
                ms_ = ds(512 * mc, 512)
                st = ps_st.tile([128, 2048], f32, tag="st", name=f"st_{mc}_{g}")
                for j in range(4):
                    nb = 4 * g + j
                    nc.tensor.matmul(st[:, ds(512 * j, 512)],
                                     k4c[nb // 4][32 * j:32 * (j + 1),
                                                  ds(128 * (nb % 4), 128)],
                                     q4c[mc][32 * j:32 * (j + 1), :],
                                     start=True, stop=True,
                                     tile_position=(32 * j, 0))
                pt = ptp.tile([128, 2048], bf16, tag="pt", name=f"pt_{mc}_{g}")
                nc.scalar.activation(pt[:], st[:], Exp)
                return pt

            # ---------- projections, with chunk-0 S^T groups slotted in ----
            # (each st group is emitted right after ~2.3us of projection PE
            # work, so its PSUM-bank wait on the previous group's exp is
            # already satisfied when the PE reaches it; limited to 3 early
            # groups by the pt pool depth)
            pre = {}
            for mc in range(MC):
                ms = ds(512 * mc, 512)
                nc.vector.tensor_copy(xr[:, :, ms], xs[:, :, ms])
                for w_t, b4, dst in ((wqT4, bq4, q4c[mc]), (wkT4, bk4, k4c[mc])):
                    pp = ps_out.tile([128, 512], f32, tag="out")
                    for o in range(2):
                        nc.tensor.matmul(pp[:], w_t[:, o, :], xr[:, o, ms],
                                         start=(o == 0), stop=(o == 1))
                    nc.scalar.activation(dst[:], pp[:], Identity, bias=b4[:])
                for nb in range(4 * mc, 4 * mc + 4):
                    pv = ps_out.tile([128, C], f32, tag="out")
                    for o in range(2):
                        nc.tensor.matmul(pv[:], xr[:, o, ds(128 * nb, 128)],
                                         wvT[:, o, :], start=(o == 0), stop=(o == 1))
                    nc.vector.tensor_copy(vTc[mc][:, nb - 4 * mc, :], pv[:])
                for o in range(2):
                    nc.vector.tensor_scalar_add(xs[:, o, ms], xs[:, o, ms],
                                                gbv[:, o:o + 1])
                if mc in (2, 4, 6):
                    pre[mc // 2 - 1] = st_group(0, mc // 2 - 1)

            pending_tail = None
            pt = None
            for mc in range(MC):
                ms = ds(512 * mc, 512)
                out_ps = [ps_out.tile([128, 512], f32, tag="out", name=f"out_{mc}_{cc}")
                          for cc in range(2)]
                s_ps = ps_misc.tile([128, 512], f32, tag="sacc")
                if pt is None:
                    pt = pre[0]
                if pending_tail is not None:
                    pending_tail()
                    pending_tail = None
                for ng in range(NG):
                    if ng + 1 < NG:
                        next_pt = (pre[ng + 1] if mc == 0 and ng + 1 in pre
                                   else st_group(mc, ng + 1))
                    elif mc + 1 < MC:
                        next_pt = st_group(mc + 1, 0)
                    else:
                        next_pt = None
                    for j in range(4):
                        nb = 4 * ng + j
                        for cc in range(2):
                            nc.tensor.matmul(out_ps[cc][:],
                                             vTc[nb // 4][:, nb % 4,
                                                          ds(128 * cc, 128)],
                                             pt[:, ds(512 * j, 512)],
                                             start=(ng == 0 and j == 0),
                                             stop=(ng == NG - 1 and j == 3))
                    for j in range(4):
                        nc.tensor.matmul(s_ps[32 * j:32 * j + 1, :], ones1[:],
                                         pt[:, ds(512 * j, 512)],
                                         start=(ng == 0), stop=(ng == NG - 1),
                                         tile_position=(0, 32 * j))
                    pt = next_pt
                out_sb = []
                for cc in range(2):
                    ob = work.tile([128, 512], f32, tag=f"ob{cc}",
                                   name=f"ob_{mc}_{cc}")
                    nc.vector.tensor_copy(ob[:], out_ps[cc][:])
                    out_sb.append(ob)
                s4c = work.tile([128, 512], f32r, tag="s4c", name=f"s4c_{mc}")
                nc.vector.tensor_copy(s4c[:], s_ps[:])

                def tail_a(mc=mc, s4c=s4c):
                    s4_sb = work.tile([4, 512], f32r, tag="s4")
                    nc.gpsimd.dma_start(s4_sb[:], s4c[0:97:32, :])
                    srep_ps = ps_misc.tile([128, 512], f32, tag="srep")
                    nc.tensor.matmul(srep_ps[:], ones4[:], s4_sb[:],
                                     start=True, stop=True)
                    r_rep = work.tile([128, 512], f32, tag="rrep")
                    nc.vector.reciprocal_approx_fast(r_rep[:], srep_ps[:])
                    nc.vector.tensor_scalar_mul(r_rep[:], r_rep[:], g_col[:])
                    return r_rep

                def tail_b(r_rep, mc=mc, out_sb=out_sb, spread=False):
                    for cc in range(2):
                        y_sb = work.tile([128, 512], f32, tag="y")
                        for h in range(2):
                            hs = ds(256 * h, 256)
                            ys = ds(512 * mc + 256 * h, 256)
                            t_sb = work.tile([128, 256], f32, tag="t")
                            nc.vector.tensor_mul(t_sb[:], out_sb[cc][:, hs],
                                                 r_rep[:, hs])
                            nc.vector.tensor_add(y_sb[:, hs], t_sb[:],
                                                 xs[:, cc, ys])
                            eng = nc.gpsimd if spread and h == 1 else nc.sync
                            eng.dma_start(y_r[:, cc, ys], y_sb[:, hs])

                def tail(mc=mc):
                    tail_b(tail_a())

                pending_tail = tail
            # last chunk: reduce-chain first (overlaps the final out-matmul
            # group + its PSUM copies), then the stores spread over two queues
            tail_b(tail_a(), spread=True)
            pending_tail = None

    nc.compile()
    return nc


def kernel(x, Wq, bq, Wk, bk, Wv, bv, gamma):
    from concourse import bass_utils

    if "nc" not in _NC_CACHE:
        _NC_CACHE["nc"] = _build_nc()
    nc = _NC_CACHE["nc"]

    x = np.ascontiguousarray(np.asarray(x, dtype=np.float32))
    shared = {
        "Wq": np.ascontiguousarray(np.asarray(Wq, dtype=np.float32)),
        "bq": np.ascontiguousarray(np.asarray(bq, dtype=np.float32)),
        "Wk": np.ascontiguousarray(np.asarray(Wk, dtype=np.float32)),
        "bk": np.ascontiguousarray(np.asarray(bk, dtype=np.float32)),
        "Wv": np.ascontiguousarray(np.asarray(Wv, dtype=np.float32)),
        "bv": np.ascontiguousarray(np.asarray(bv, dtype=np.float32)),
        "gamma": np.ascontiguousarray(np.asarray(gamma, dtype=np.float32)),
    }
    in_maps = [dict(shared, x=np.ascontiguousarray(x[i].reshape(C, N)))
               for i in range(B)]

    res = bass_utils.run_bass_kernel_spmd(nc, in_maps, core_ids=list(range(B)))
    y = np.stack([res.results[i]["y"] for i in range(B)], axis=0)
    return y.reshape(B, C, H, W).astype(np.float32)
